# revision 23
# baseline (speedup 1.0000x reference)
"""Causal multi-head attention on 8 Trainium2 NeuronCores.

Sharding: Megatron-style tensor parallelism over heads. Each of the 8
cores computes 2 of the 16 heads end-to-end:
  - column-parallel Q/K/V projections (each core owns a 256-feature slice
    of wq/wk/wv),
  - per-head causal attention entirely on-core,
  - row-parallel output projection producing a partial [B*S, D] output.
The 8 partials are summed on the host (the "all-reduce") and bo added.

Device layout notes:
  - All matmuls run as float32r (full-rate fp32 on the PE at N>=256).
  - Activations are kept feature-major (transposed): qT/kT are [hd, S]
    per head, so scoresT = kT.T-free matmul with q as the moving operand,
    softmax runs in the transposed layout (sums via a ones-matmul), and
    probs feed the PV matmul directly as the moving operand — the kernel
    contains zero on-chip transposes.
  - Causality is exploited at block granularity: upper-triangle score
    blocks are never computed; diagonal blocks get a static additive mask.
"""

import math

import numpy as np

B = 2
S = 2048
D = 2048
H = 16
HD = 128  # head dim
N_CORES = 8
H_LOC = H // N_CORES       # 2 heads per core
F_LOC = H_LOC * HD         # 256 local features per core
KT = D // 128              # 16 contraction tiles
CHUNK = 512                # token chunk (matmul moving dim)
NCH = S // CHUNK           # 4 chunks per batch
TT = S // 128              # 16 token tiles per batch

_CACHE = {}


def _build():
    import concourse.bass as bass
    import concourse.mybir as mybir
    import concourse.tile as tile
    from concourse import bacc

    F32 = mybir.dt.float32
    F32R = mybir.dt.float32r
    ADD = mybir.AluOpType.add
    MULT = mybir.AluOpType.mult
    EXP = mybir.ActivationFunctionType.Exp
    INV_SQRT_HD = 1.0 / math.sqrt(HD)

    nc = bacc.Bacc("TRN2", target_bir_lowering=False, debug=False,
                   num_devices=N_CORES)

    xT_d = nc.dram_tensor("xT", [D, B * S], F32R, kind="ExternalInput")
    wqT_d = nc.dram_tensor("wqT", [D, F_LOC], F32R, kind="ExternalInput")
    wkT_d = nc.dram_tensor("wkT", [D, F_LOC], F32R, kind="ExternalInput")
    wvT_d = nc.dram_tensor("wvT", [D, F_LOC], F32R, kind="ExternalInput")
    woT_d = nc.dram_tensor("woT", [F_LOC, D], F32R, kind="ExternalInput")
    bq_d = nc.dram_tensor("bq2", [HD, H_LOC], F32, kind="ExternalInput")
    bk_d = nc.dram_tensor("bk2", [HD, H_LOC], F32, kind="ExternalInput")
    bv_d = nc.dram_tensor("bv1", [1, F_LOC], F32R, kind="ExternalInput")
    ones_d = nc.dram_tensor("ones", [128, 128], F32R, kind="ExternalInput")
    y_d = nc.dram_tensor("y", [B * S, D], F32, kind="ExternalOutput")

    with tile.TileContext(nc) as tc:
        cpool = tc.alloc_tile_pool(name="const", bufs=1)
        wpool = tc.alloc_tile_pool(name="w", bufs=1)
        xkpool = tc.alloc_tile_pool(name="xk", bufs=2)
        actpool = tc.alloc_tile_pool(name="act", bufs=4)
        ypool = tc.alloc_tile_pool(name="y", bufs=3)
        ripool = tc.alloc_tile_pool(name="ri", bufs=2)
        ps = tc.alloc_tile_pool(name="ps", bufs=5, space="PSUM")
        ps_out = tc.alloc_tile_pool(name="pso", bufs=3, space="PSUM")

        # --- constants + weights ---
        # DMA queue order is deliberate: wq then the first x chunk so the
        # first projection matmuls start ~17us in; remaining weights follow
        # and arrive before their first consumers.
        def load_w(nm, dram):
            w_t = wpool.tile([128, KT * F_LOC], F32R, tag=nm)
            nc.sync.dma_start(
                w_t[:].rearrange("p (k f) -> p k f", k=KT),
                dram.ap().rearrange("(k p) f -> p k f", p=128),
            )
            return w_t

        def load_x(b, c, split=1):
            x_t = xkpool.tile([128, KT * CHUNK], F32R, tag="xk")
            col0 = b * S + c * CHUNK
            kstep = KT // split
            for s in range(split):
                k0 = s * kstep
                nc.sync.dma_start(
                    x_t[:, k0 * CHUNK:(k0 + kstep) * CHUNK]
                        .rearrange("p (k f) -> p k f", k=kstep),
                    xT_d.ap()[k0 * 128:(k0 + kstep) * 128,
                              col0:col0 + CHUNK]
                        .rearrange("(k p) f -> p k f", p=128),
                )
            return x_t

        w_ts = {"wq": load_w("wq", wqT_d)}
        x_first = load_x(0, 0, split=4)
        w_ts["wk"] = load_w("wk", wkT_d)
        w_ts["wv"] = load_w("wv", wvT_d)
        x_second = load_x(0, 1, split=2)
        bq_t = cpool.tile([HD, H_LOC], F32, tag="bq")
        bk_t = cpool.tile([HD, H_LOC], F32, tag="bk")
        bv_t = cpool.tile([1, F_LOC], F32R, tag="bv")
        nc.sync.dma_start(bq_t[:], bq_d.ap())
        nc.sync.dma_start(bk_t[:], bk_d.ap())
        nc.sync.dma_start(bv_t[:], bv_d.ap())
        ones128 = cpool.tile([128, 128], F32R, tag="ones128")
        nc.sync.dma_start(ones128[:], ones_d.ap())
        ones1 = ones128
        maskT = cpool.tile([128, 128], F32, tag="maskT")
        nc.gpsimd.memset(maskT[:], 0.0)
        # transposed causal mask: keep (0) where k_part <= q_free else -1e9
        nc.gpsimd.affine_select(
            out=maskT[:], in_=maskT[:],
            compare_op=mybir.AluOpType.is_ge,
            fill=-1e9, base=0, pattern=[[1, 128]], channel_multiplier=-1,
        )

        x_pref = x_first
        wo_t = None
        for b in range(B):
            # ---------------- QKV projections for batch b ----------------
            qT_t = actpool.tile([128, H_LOC * S], F32R, tag="act")
            kT_t = actpool.tile([128, H_LOC * S], F32R, tag="act")
            v_t = actpool.tile([128, TT * F_LOC], F32R, tag="act")
            for c in range(NCH):
                if c == 0:
                    x_t = x_pref
                elif b == 0 and c == 1:
                    x_t = x_second
                else:
                    x_t = load_x(b, c)
                if wo_t is None and c == NCH - 1:
                    # woT [F_LOC, D] -> [128, H_LOC*D]; deferred so the DMA
                    # queue prioritizes x chunks during warmup
                    wo_t = wpool.tile([128, H_LOC * D], F32R, tag="wo")
                    nc.sync.dma_start(
                        wo_t[:].rearrange("p (h f) -> p h f", h=H_LOC),
                        woT_d.ap().rearrange("(h p) f -> p h f", p=128),
                    )
                # qT / kT: out[hd, tok] accumulated over k-tiles
                for nm, dst, bias in (("wq", qT_t, bq_t), ("wk", kT_t, bk_t)):
                    for h in range(H_LOC):
                        q_ps = ps.tile([128, CHUNK], F32, tag="ps")
                        for k in range(KT):
                            nc.tensor.matmul(
                                q_ps[:],
                                w_ts[nm][:, k * F_LOC + h * HD:
                                         k * F_LOC + (h + 1) * HD],
                                x_t[:, k * CHUNK:(k + 1) * CHUNK],
                                start=(k == 0), stop=(k == KT - 1),
                            )
                        nc.vector.tensor_scalar_add(
                            dst[:, h * S + c * CHUNK: h * S + (c + 1) * CHUNK],
                            q_ps[:], bias[:, h:h + 1])
                # v: natural layout [tok, f_loc], 128-token tiles
                for t4 in range(CHUNK // 128):
                    tt = c * (CHUNK // 128) + t4
                    v_ps = ps.tile([128, CHUNK], F32, tag="ps")
                    for k in range(KT):
                        nc.tensor.matmul(
                            v_ps[:, 0:F_LOC],
                            x_t[:, k * CHUNK + t4 * 128:
                                k * CHUNK + (t4 + 1) * 128],
                            w_ts["wv"][:, k * F_LOC:(k + 1) * F_LOC],
                            start=(k == 0), stop=False,
                        )
                    nc.tensor.matmul(  # + bv via rank-1 ones
                        v_ps[:, 0:F_LOC],
                        ones1[0:1, 0:128], bv_t[:],
                        start=False, stop=True,
                    )
                    nc.vector.tensor_copy(
                        v_t[:, tt * F_LOC:(tt + 1) * F_LOC], v_ps[:, 0:F_LOC])

            # -------- attention + output projection, chunk-interleaved ------
            attnT_t = actpool.tile([128, H_LOC * S], F32R, tag="act")
            for c in range(NCH):
                for h in range(H_LOC):
                    nki = 4 * c + 4
                    e_t = xkpool.tile([128, KT * CHUNK], F32R, tag="xk")
                    # scoresT blocks + exp (transposed layout: [k, q])
                    for ki in range(nki):
                        r = ki - 4 * c
                        trim = 128 * r if r > 0 else 0
                        ncol = CHUNK - trim
                        s_ps = ps.tile([128, CHUNK], F32, tag="ps")
                        nc.tensor.matmul(
                            s_ps[:, 0:ncol],
                            kT_t[:, h * S + ki * 128:
                                 h * S + (ki + 1) * 128],
                            qT_t[:, h * S + c * CHUNK + trim:
                                 h * S + (c + 1) * CHUNK],
                            start=True, stop=True,
                        )
                        if ki >= 4 * c:  # diagonal 128x128 needs the mask
                            nc.vector.tensor_tensor(
                                s_ps[:, 0:128], s_ps[:, 0:128], maskT[:], ADD)
                        nc.scalar.activation(
                            e_t[:, ki * CHUNK + trim:(ki + 1) * CHUNK],
                            s_ps[:, 0:ncol], EXP, bias=0.0, scale=INV_SQRT_HD)
                    # PV accumulation over ki (PE)
                    at_ps = ps.tile([128, CHUNK], F32, tag="ps")
                    for ki in range(nki):
                        r = ki - 4 * c
                        trim = 128 * r if r > 0 else 0
                        nc.tensor.matmul(
                            at_ps[:, trim:CHUNK],
                            v_t[:, ki * F_LOC + h * HD:
                                ki * F_LOC + (h + 1) * HD],
                            e_t[:, ki * CHUNK + trim:
                                (ki + 1) * CHUNK],
                            start=(ki == 0), stop=(ki == nki - 1),
                        )
                    # rowsum over partitions via ones-matmul (replicated rows)
                    rs_ps = ps.tile([128, CHUNK], F32, tag="ps")
                    for ki in range(nki):
                        r = ki - 4 * c
                        trim = 128 * r if r > 0 else 0
                        nc.tensor.matmul(
                            rs_ps[:, trim:CHUNK],
                            ones128[:],
                            e_t[:, ki * CHUNK + trim:
                                (ki + 1) * CHUNK],
                            start=(ki == 0), stop=(ki == nki - 1),
                        )
                    ri_t = ripool.tile([128, CHUNK], F32, tag="ri")
                    nc.vector.reciprocal(ri_t[:], rs_ps[:])
                    nc.vector.tensor_tensor(
                        attnT_t[:, h * S + c * CHUNK: h * S + (c + 1) * CHUNK],
                        at_ps[:], ri_t[:], MULT)

                if c == 0 and b + 1 < B:
                    # prefetch next batch's first x chunk ahead of the y
                    # write burst below
                    x_pref = load_x(b + 1, 0, split=2)

                # output projection for this chunk's 4 token tiles
                for t4 in range(CHUNK // 128):
                    tt = c * (CHUNK // 128) + t4
                    for oc in range(D // CHUNK):
                        o_ps = ps_out.tile([128, CHUNK], F32, tag="pso")
                        for h in range(H_LOC):
                            nc.tensor.matmul(
                                o_ps[:],
                                attnT_t[:, h * S + tt * 128:
                                        h * S + (tt + 1) * 128],
                                wo_t[:, h * D + oc * CHUNK:
                                     h * D + (oc + 1) * CHUNK],
                                start=(h == 0), stop=(h == H_LOC - 1),
                            )
                        y_t = ypool.tile([128, CHUNK], F32, tag="y")
                        nc.vector.tensor_copy(y_t[:], o_ps[:])
                        row0 = b * S + tt * 128
                        nc.sync.dma_start(
                            y_d.ap()[row0:row0 + 128,
                                     oc * CHUNK:(oc + 1) * CHUNK], y_t[:])

        for p in (ps_out, ps, ripool, ypool, actpool, xkpool, wpool, cpool):
            p.release()

    nc.compile()
    return nc


def _get_nc():
    if "nc" not in _CACHE:
        _CACHE["nc"] = _build()
    return _CACHE["nc"]


def kernel(x, wq, bq, wk, bk, wv, bv, wo, bo):
    from concourse.bass_utils import run_bass_kernel_spmd

    nc = _get_nc()

    x = np.asarray(x, dtype=np.float32)
    xT = np.ascontiguousarray(x.reshape(B * S, D).T)  # [D, B*S]

    in_maps = []
    for i in range(N_CORES):
        fs = slice(i * F_LOC, (i + 1) * F_LOC)
        in_maps.append({
            "xT": xT,
            "wqT": np.ascontiguousarray(np.asarray(wq)[fs, :].T),
            "wkT": np.ascontiguousarray(np.asarray(wk)[fs, :].T),
            "wvT": np.ascontiguousarray(np.asarray(wv)[fs, :].T),
            "woT": np.ascontiguousarray(np.asarray(wo)[:, fs].T),
            "bq2": np.ascontiguousarray(
                np.asarray(bq)[fs].reshape(H_LOC, HD).T),
            "bk2": np.ascontiguousarray(
                np.asarray(bk)[fs].reshape(H_LOC, HD).T),
            "bv1": np.ascontiguousarray(
                np.asarray(bv)[fs].reshape(1, F_LOC)),
            "ones": np.ones((128, 128), dtype=np.float32),
        })

    res = run_bass_kernel_spmd(nc, in_maps, core_ids=list(range(N_CORES)),
                               trace=False)
    y = np.zeros((B * S, D), dtype=np.float32)
    for i in range(N_CORES):
        y += res.results[i]["y"]
    y += np.asarray(bo, dtype=np.float32)[None, :]
    return y.reshape(B, S, D)
